# revision 10
# baseline (speedup 1.0000x reference)
"""Trainium2 Bass kernel for nn_BatchRelationalModule.

Math (per batch element, see reference):
  featsT = [x_img[b].reshape(64, 256); arange(256)]            # [65, 256]
  pair MLP layer 0 decomposes: Wg0 @ concat(f_q, f_p) = u[:,q] + v[:,p]
    u = Wg0[:, :65] @ featsT + bg0, v = Wg0[:, 65:] @ featsT
  X0[p,q] = relu(u[:,q] + v[:,p])                              # 256x256 pairs
  X1 = relu(Wg1 @ X0 + bg1); X2 = relu(Wg2 @ X1 + bg2)
  S = sum_{p,q} X2;  out = Wo @ relu(Wp @ S + bp) + bo

Device layout: features (64) on partitions, pairs on free dim.  Two p-blocks
(p and p+128) are stacked to fill 128 partitions; block-diagonal [128,128]
weights process both halves per matmul.  ACT accum_out produces the row-sums
of X2 for free; the final Wp matmul (K=128) folds the two halves.

Sharding: data-parallel over batch - 16 batches / 8 cores = 2 per core,
weights replicated, outputs gathered on host.
"""

import os
from contextlib import ExitStack

import numpy as np

import concourse.bass as bass
import concourse.tile as tile
from concourse import bacc, mybir
from concourse.bass_utils import run_bass_kernel_spmd

F32 = mybir.dt.float32
N_CORES = 8
B_PER_CORE = 2
L = 256  # h*w
C = 64
F = 64
D = C + 1  # 65

_CACHE = {}


def _build_nc(repeat=1):
    nc = bacc.Bacc(
        "TRN2",
        target_bir_lowering=False,
        debug=False,
        enable_asserts=False,
        num_devices=N_CORES,
    )

    # DRAM tensors (per-core inputs)
    xf = nc.dram_tensor("xf", [B_PER_CORE, C, L], F32, kind="ExternalInput").ap()
    coord = nc.dram_tensor("coord", [1, L], F32, kind="ExternalInput").ap()
    wg0lT_dd = nc.dram_tensor("wg0lT_dd", [D, 128], F32, kind="ExternalInput").ap()
    wg0rT_dd = nc.dram_tensor("wg0rT_dd", [D, 128], F32, kind="ExternalInput").ap()
    bg0dd = nc.dram_tensor("bg0dd", [128, 1], F32, kind="ExternalInput").ap()
    w1dd = nc.dram_tensor("w1dd", [128, F], F32, kind="ExternalInput").ap()
    w2dd = nc.dram_tensor("w2dd", [128, F], F32, kind="ExternalInput").ap()
    bg1dd = nc.dram_tensor("bg1dd", [128, 1], F32, kind="ExternalInput").ap()
    bg2dd = nc.dram_tensor("bg2dd", [128, 1], F32, kind="ExternalInput").ap()
    wpT_dd = nc.dram_tensor("wpT_dd", [128, F], F32, kind="ExternalInput").ap()
    bp_c = nc.dram_tensor("bp_c", [F, 1], F32, kind="ExternalInput").ap()
    woT = nc.dram_tensor("woT", [F, F], F32, kind="ExternalInput").ap()
    bo_c = nc.dram_tensor("bo_c", [F, 1], F32, kind="ExternalInput").ap()
    out = nc.dram_tensor("out", [B_PER_CORE, F, 1], F32, kind="ExternalOutput").ap()

    add = mybir.AluOpType.add
    mx = mybir.AluOpType.max
    Relu = mybir.ActivationFunctionType.Relu
    Ident = mybir.ActivationFunctionType.Identity

    with tile.TileContext(nc) as tc, ExitStack() as ctx:
        consts = ctx.enter_context(tc.tile_pool(name="consts", bufs=1))
        setup = ctx.enter_context(tc.tile_pool(name="setup", bufs=2))
        xp = ctx.enter_context(tc.tile_pool(name="xp", bufs=3))
        scratch = ctx.enter_context(tc.tile_pool(name="scratch", bufs=2))
        ps1p = ctx.enter_context(tc.tile_pool(name="ps1p", bufs=2, space="PSUM"))
        ps2p = ctx.enter_context(tc.tile_pool(name="ps2p", bufs=2, space="PSUM"))
        accp = ctx.enter_context(tc.tile_pool(name="accp", bufs=2))
        pssp = ps1p  # setup-phase psum shares ps1 slots (tag below)

        def load_const(name, ap_in, shape):
            t = consts.tile(shape, F32, name=name)
            nc.sync.dma_start(t[:], ap_in)
            return t

        wg0lT_sb = load_const("wg0lT_sb", wg0lT_dd, [D, 128])
        wg0rT_sb = load_const("wg0rT_sb", wg0rT_dd, [D, 128])
        bg0dd_sb = load_const("bg0dd_sb", bg0dd, [128, 1])
        w1dd_sb = load_const("w1dd_sb", w1dd, [128, F])
        w2dd_sb = load_const("w2dd_sb", w2dd, [128, F])
        bg1dd_sb = load_const("bg1dd_sb", bg1dd, [128, 1])
        bg2dd_sb = load_const("bg2dd_sb", bg2dd, [128, 1])
        wpT_dd_sb = load_const("wpT_dd_sb", wpT_dd, [128, F])
        bp_sb = load_const("bp_sb", bp_c, [F, 1])
        woT_sb = load_const("woT_sb", woT, [F, F])
        bo_sb = load_const("bo_sb", bo_c, [F, 1])

        def body():
            _emit_body(
                nc, tc, setup, xp, scratch, ps1p, ps2p, pssp, accp,
                xf, coord, out,
                wg0lT_sb, wg0rT_sb, bg0dd_sb, w1dd_sb, w2dd_sb,
                bg1dd_sb, bg2dd_sb, wpT_dd_sb, bp_sb, woT_sb, bo_sb,
            )

        if repeat == 1:
            body()
        else:
            hint = (
                mybir.EngineType.PE,
                mybir.EngineType.DVE,
                mybir.EngineType.Activation,
                mybir.EngineType.SP,
                mybir.EngineType.Pool,
            )
            with tc.For_i(0, repeat, 1, hint_engines=hint):
                body()

    nc.compile()
    return nc


def _emit_body(
    nc, tc, setup, xp, scratch, ps1p, ps2p, pssp, accp,
    xf, coord, out,
    wg0lT_sb, wg0rT_sb, bg0dd_sb, w1dd_sb, w2dd_sb,
    bg1dd_sb, bg2dd_sb, wpT_dd_sb, bp_sb, woT_sb, bo_sb,
):
    add = mybir.AluOpType.add
    mx = mybir.AluOpType.max
    Relu = mybir.ActivationFunctionType.Relu
    Ident = mybir.ActivationFunctionType.Identity

    FD = 1024          # free dim of the main tiles (4 x 256-col p-blocks)
    NB = FD // L       # p-blocks per half per iteration (4)
    NITER = 128 // NB  # iterations per batch (32)

    if True:
        for b in range(B_PER_CORE):
            featsT = setup.tile([D, L], F32, name="featsT", tag="featsT")
            nc.sync.dma_start(featsT[0:C, :], xf[b])
            nc.sync.dma_start(featsT[C : C + 1, :], coord)

            # u (duplicated on both partition halves by the M=128 stationary)
            ps_u = pssp.tile([128, L], F32, name="ps_u", tag="ps1")
            nc.tensor.matmul(ps_u[:], wg0lT_sb[:], featsT[:], start=True, stop=True)
            udup = setup.tile([128, L], F32, name="udup", tag="udup")
            nc.scalar.activation(udup[:], ps_u[:], Ident, bias=bg0dd_sb[:])

            ps_v = pssp.tile([128, L], F32, name="ps_v", tag="ps1")
            nc.tensor.matmul(ps_v[:], wg0rT_sb[:], featsT[:], start=True, stop=True)
            # v2[:, i] = [v[:, i] (top) ; v[:, 128+i] (bottom)]
            v2 = setup.tile([128, 128], F32, name="v2", tag="v2")
            nc.vector.tensor_copy(v2[0:64, :], ps_v[0:64, 0:128])
            nc.vector.tensor_copy(v2[64:128, :], ps_v[64:128, 128:256])

            acc = accp.tile([128, NITER], F32, name="acc", tag="acc")

            for i in range(NITER):
                # X0 = relu(u + v_p); block k covers p = 32k+i (top),
                # 128+32k+i (bottom)
                x0 = xp.tile([128, FD], F32, name="x0", tag="x0")
                for k in range(NB):
                    nc.vector.tensor_scalar(
                        x0[:, k * L : (k + 1) * L], udup[:],
                        v2[:, NITER * k + i : NITER * k + i + 1],
                        0.0, op0=add, op1=mx,
                    )
                # layer 1: 2x2 quadrant-packed 64x64 matmuls
                ps1 = ps1p.tile([128, FD], F32, name="ps1", tag="ps1")
                for c in range(FD // 512):
                    cs = slice(512 * c, 512 * (c + 1))
                    nc.tensor.matmul(
                        ps1[0:64, cs], w1dd_sb[0:64, :], x0[0:64, cs],
                        start=True, stop=True,
                    )
                    nc.tensor.matmul(
                        ps1[64:128, cs], w1dd_sb[64:128, :], x0[64:128, cs],
                        start=True, stop=True,
                    )
                # X1 = relu(ps1 + bg1): alternate DVE / ACT for balance
                x1 = xp.tile([128, FD], F32, name="x1", tag="x1")
                if i % 2 == 0:
                    nc.scalar.activation(x1[:], ps1[:], Relu, bias=bg1dd_sb[:])
                else:
                    nc.vector.tensor_scalar(
                        x1[:], ps1[:], bg1dd_sb[:], 0.0, op0=add, op1=mx
                    )
                # layer 2 (note: output halves land swapped; harmless for sum)
                ps2 = ps2p.tile([128, FD], F32, name="ps2", tag="ps2")
                for c in range(FD // 512):
                    cs = slice(512 * c, 512 * (c + 1))
                    nc.tensor.matmul(
                        ps2[64:128, cs], w2dd_sb[0:64, :], x1[0:64, cs],
                        start=True, stop=True,
                    )
                    nc.tensor.matmul(
                        ps2[0:64, cs], w2dd_sb[64:128, :], x1[64:128, cs],
                        start=True, stop=True,
                    )
                # X2 = relu(ps2 + bg2) on ACT; accum_out -> row sums
                x2 = scratch.tile([128, FD], F32, name="x2", tag="x2")
                nc.scalar.activation(
                    x2[:], ps2[:], Relu, bias=bg2dd_sb[:],
                    accum_out=acc[:, i : i + 1],
                )

            # Reduce accumulated columns -> [128, 1]
            accr = setup.tile([128, 1], F32, name="accr", tag="accr")
            nc.vector.tensor_reduce(
                accr[:], acc[:], axis=mybir.AxisListType.X, op=add
            )
            # f-network; K=128 matmul folds top+bottom halves of accr
            ps_h = pssp.tile([F, 1], F32, name="ps_h", tag="ps1")
            nc.tensor.matmul(ps_h[:], wpT_dd_sb[:], accr[:], start=True, stop=True)
            h_sb = setup.tile([F, 1], F32, name="h_sb", tag="h_sb")
            nc.scalar.activation(h_sb[:], ps_h[:], Relu, bias=bp_sb[:])
            ps_o = pssp.tile([F, 1], F32, name="ps_o", tag="ps1")
            nc.tensor.matmul(ps_o[:], woT_sb[:], h_sb[:], start=True, stop=True)
            o_sb = setup.tile([F, 1], F32, name="o_sb", tag="o_sb")
            nc.scalar.activation(o_sb[:], ps_o[:], Ident, bias=bo_sb[:])
            nc.sync.dma_start(out[b], o_sb[:])


def _shared_in_map(Wg0, bg0, Wg1, bg1, Wg2, bg2, Wp, bp, Wo, bo):
    f = np.float32
    wg0l = np.ascontiguousarray(Wg0[:, :D].T, dtype=f)  # [65, 64]
    wg0r = np.ascontiguousarray(Wg0[:, D:].T, dtype=f)  # [65, 64]
    stackT = lambda w: np.concatenate(
        [np.ascontiguousarray(w.T, dtype=f)] * 2, axis=0
    )
    return {
        "coord": np.arange(L, dtype=f).reshape(1, L),
        "wg0lT_dd": np.concatenate([wg0l, wg0l], axis=1),
        "wg0rT_dd": np.concatenate([wg0r, wg0r], axis=1),
        "bg0dd": np.concatenate([bg0, bg0]).astype(f).reshape(128, 1),
        "w1dd": stackT(Wg1),
        "w2dd": stackT(Wg2),
        "bg1dd": np.concatenate([bg1, bg1]).astype(f).reshape(128, 1),
        "bg2dd": np.concatenate([bg2, bg2]).astype(f).reshape(128, 1),
        "wpT_dd": np.concatenate([Wp.T, Wp.T], axis=0).astype(f),
        "bp_c": np.asarray(bp, f).reshape(F, 1),
        "woT": np.ascontiguousarray(Wo.T, dtype=f),
        "bo_c": np.asarray(bo, f).reshape(F, 1),
    }


def kernel(
    x_img, Wg0, bg0, Wg1, bg1, Wg2, bg2, Wp, bp, Wo, bo, trace=False, **run_kwargs
):
    if "nc" not in _CACHE:
        _CACHE["nc"] = _build_nc()
    nc = _CACHE["nc"]

    shared = _shared_in_map(
        np.asarray(Wg0), np.asarray(bg0), np.asarray(Wg1), np.asarray(bg1),
        np.asarray(Wg2), np.asarray(bg2), np.asarray(Wp), np.asarray(bp),
        np.asarray(Wo), np.asarray(bo),
    )
    x = np.asarray(x_img, dtype=np.float32)
    bsz = x.shape[0]
    x = x.reshape(bsz, C, L)

    in_maps = []
    for core in range(N_CORES):
        m = dict(shared)
        m["xf"] = np.ascontiguousarray(x[core * B_PER_CORE : (core + 1) * B_PER_CORE])
        in_maps.append(m)

    res = run_bass_kernel_spmd(
        nc, in_maps, core_ids=list(range(N_CORES)), trace=trace, **run_kwargs
    )
    outs = [r["out"].reshape(B_PER_CORE, F) for r in res.results]
    full = np.concatenate(outs, axis=0)
    if trace:
        _CACHE["last_results"] = res
    return full


# revision 16
# speedup vs baseline: 4.3146x; 4.3146x over previous
"""Trainium2 Bass kernel for nn_BatchRelationalModule.

Math (per batch element, see reference):
  featsT = [x_img[b].reshape(64, 256); arange(256)]            # [65, 256]
  pair MLP layer 0 decomposes: Wg0 @ concat(f_q, f_p) = u[:,q] + v[:,p]
    u = Wg0[:, :65] @ featsT + bg0, v = Wg0[:, 65:] @ featsT
  X0[p,q] = relu(u[:,q] + v[:,p])                              # 256x256 pairs
  X1 = relu(Wg1 @ X0 + bg1); X2 = relu(Wg2 @ X1 + bg2)
  S = sum_{p,q} X2;  out = Wo @ relu(Wp @ S + bp) + bo

Device layout: features (64) on partitions, pairs on free dim.  Two p-blocks
(p and p+128) are stacked to fill 128 partitions; block-diagonal [128,128]
weights process both halves per matmul.  ACT accum_out produces the row-sums
of X2 for free; the final Wp matmul (K=128) folds the two halves.

Sharding: data-parallel over batch - 16 batches / 8 cores = 2 per core,
weights replicated, outputs gathered on host.
"""

import os
from contextlib import ExitStack

import numpy as np

import concourse.bass as bass
import concourse.tile as tile
from concourse import bacc, mybir
from concourse.bass_utils import run_bass_kernel_spmd

F32 = mybir.dt.float32
F16 = mybir.dt.float16
N_CORES = 8
B_PER_CORE = 2
L = 256  # h*w
C = 64
F = 64
D = C + 1  # 65

_CACHE = {}


def _build_nc(repeat=1):
    nc = bacc.Bacc(
        "TRN2",
        target_bir_lowering=False,
        debug=False,
        enable_asserts=False,
        num_devices=N_CORES,
    )

    # DRAM tensors (per-core inputs)
    xf = nc.dram_tensor("xf", [B_PER_CORE, C, L], F32, kind="ExternalInput").ap()
    coord = nc.dram_tensor("coord", [1, L], F32, kind="ExternalInput").ap()
    wg0lT_dd = nc.dram_tensor("wg0lT_dd", [D, 128], F32, kind="ExternalInput").ap()
    wg0rT_dd = nc.dram_tensor("wg0rT_dd", [D, 128], F32, kind="ExternalInput").ap()
    bg0dd = nc.dram_tensor("bg0dd", [128, 1], F32, kind="ExternalInput").ap()
    w1dd = nc.dram_tensor("w1dd", [128, F], F16, kind="ExternalInput").ap()
    w2dd = nc.dram_tensor("w2dd", [128, F], F16, kind="ExternalInput").ap()
    bg1dd = nc.dram_tensor("bg1dd", [128, 1], F32, kind="ExternalInput").ap()
    bg2dd = nc.dram_tensor("bg2dd", [128, 1], F32, kind="ExternalInput").ap()
    wpT_dd = nc.dram_tensor("wpT_dd", [128, F], F32, kind="ExternalInput").ap()
    bp_c = nc.dram_tensor("bp_c", [F, 1], F32, kind="ExternalInput").ap()
    woT = nc.dram_tensor("woT", [F, F], F32, kind="ExternalInput").ap()
    bo_c = nc.dram_tensor("bo_c", [F, 1], F32, kind="ExternalInput").ap()
    out = nc.dram_tensor("out", [B_PER_CORE, F, 1], F32, kind="ExternalOutput").ap()

    add = mybir.AluOpType.add
    mx = mybir.AluOpType.max
    Relu = mybir.ActivationFunctionType.Relu
    Ident = mybir.ActivationFunctionType.Identity

    with tile.TileContext(nc) as tc, ExitStack() as ctx:
        consts = ctx.enter_context(tc.tile_pool(name="consts", bufs=1))
        setup = ctx.enter_context(tc.tile_pool(name="setup", bufs=2))
        xp = ctx.enter_context(tc.tile_pool(name="xp", bufs=3))
        scratch = ctx.enter_context(tc.tile_pool(name="scratch", bufs=2))
        ps1p = ctx.enter_context(tc.tile_pool(name="ps1p", bufs=2, space="PSUM"))
        ps2p = ctx.enter_context(tc.tile_pool(name="ps2p", bufs=2, space="PSUM"))
        accp = ctx.enter_context(tc.tile_pool(name="accp", bufs=2))
        pssp = ps1p  # setup-phase psum shares ps1 slots (tag below)

        def load_const(name, ap_in, shape, dt=F32):
            t = consts.tile(shape, dt, name=name)
            nc.sync.dma_start(t[:], ap_in)
            return t

        wg0lT_sb = load_const("wg0lT_sb", wg0lT_dd, [D, 128])
        wg0rT_sb = load_const("wg0rT_sb", wg0rT_dd, [D, 128])
        bg0dd_sb = load_const("bg0dd_sb", bg0dd, [128, 1])
        w1dd_sb = load_const("w1dd_sb", w1dd, [128, F], F16)
        w2dd_sb = load_const("w2dd_sb", w2dd, [128, F], F16)
        bg1dd_sb = load_const("bg1dd_sb", bg1dd, [128, 1])
        bg2dd_sb = load_const("bg2dd_sb", bg2dd, [128, 1])
        wpT_dd_sb = load_const("wpT_dd_sb", wpT_dd, [128, F])
        bp_sb = load_const("bp_sb", bp_c, [F, 1])
        woT_sb = load_const("woT_sb", woT, [F, F])
        bo_sb = load_const("bo_sb", bo_c, [F, 1])

        def body():
            _emit_body(
                nc, tc, setup, xp, scratch, ps1p, ps2p, pssp, accp,
                xf, coord, out,
                wg0lT_sb, wg0rT_sb, bg0dd_sb, w1dd_sb, w2dd_sb,
                bg1dd_sb, bg2dd_sb, wpT_dd_sb, bp_sb, woT_sb, bo_sb,
            )

        if repeat == 1:
            body()
        else:
            hint = (
                mybir.EngineType.PE,
                mybir.EngineType.DVE,
                mybir.EngineType.Activation,
                mybir.EngineType.SP,
                mybir.EngineType.Pool,
            )
            with tc.For_i(0, repeat, 1, hint_engines=hint):
                body()

    nc.compile()
    return nc


def _emit_body(
    nc, tc, setup, xp, scratch, ps1p, ps2p, pssp, accp,
    xf, coord, out,
    wg0lT_sb, wg0rT_sb, bg0dd_sb, w1dd_sb, w2dd_sb,
    bg1dd_sb, bg2dd_sb, wpT_dd_sb, bp_sb, woT_sb, bo_sb,
):
    add = mybir.AluOpType.add
    mx = mybir.AluOpType.max
    Relu = mybir.ActivationFunctionType.Relu
    Ident = mybir.ActivationFunctionType.Identity

    FD = 1024          # free dim of the main tiles (4 x 256-col p-blocks)
    NB = FD // L       # p-blocks per half per iteration (4)
    NITER = 128 // NB  # iterations per batch (32)

    if True:
        for b in range(B_PER_CORE):
            featsT = setup.tile([D, L], F32, name="featsT", tag="featsT")
            nc.sync.dma_start(featsT[0:C, :], xf[b])
            nc.sync.dma_start(featsT[C : C + 1, :], coord)

            # u (duplicated on both partition halves by the M=128 stationary)
            ps_u = pssp.tile([128, L], F32, name="ps_u", tag="ps1")
            nc.tensor.matmul(ps_u[:], wg0lT_sb[:], featsT[:], start=True, stop=True)
            udup = setup.tile([128, L], F16, name="udup", tag="udup")
            nc.scalar.activation(udup[:], ps_u[:], Ident, bias=bg0dd_sb[:])

            ps_v = pssp.tile([128, L], F32, name="ps_v", tag="ps1")
            nc.tensor.matmul(ps_v[:], wg0rT_sb[:], featsT[:], start=True, stop=True)
            # v2[:, i] = [v[:, i] (top) ; v[:, 128+i] (bottom)]  (fp32 scalars)
            v2 = setup.tile([128, 128], F32, name="v2", tag="v2")
            nc.vector.tensor_copy(v2[0:64, :], ps_v[0:64, 0:128])
            nc.vector.tensor_copy(v2[64:128, :], ps_v[64:128, 128:256])

            acc = accp.tile([128, NITER], F32, name="acc", tag="acc")

            for i in range(NITER):
                # X0 = relu(u + v_p); block k covers p = 32k+i (top),
                # 128+32k+i (bottom)
                x0 = xp.tile([128, FD], F16, name="x0", tag="x0")
                for k in range(NB):
                    nc.vector.tensor_scalar(
                        x0[:, k * L : (k + 1) * L], udup[:],
                        v2[:, NITER * k + i : NITER * k + i + 1],
                        0.0, op0=add, op1=mx,
                    )
                # layer 1: 2x2 quadrant-packed 64x64 fp16 matmuls
                ps1 = ps1p.tile([128, FD], F32, name="ps1", tag="ps1")
                for c in range(FD // 512):
                    cs = slice(512 * c, 512 * (c + 1))
                    nc.tensor.matmul(
                        ps1[0:64, cs], w1dd_sb[0:64, :], x0[0:64, cs],
                        start=True, stop=True,
                    )
                    nc.tensor.matmul(
                        ps1[64:128, cs], w1dd_sb[64:128, :], x0[64:128, cs],
                        start=True, stop=True,
                    )
                # X1 = relu(ps1 + bg1): DVE 2 of 3 iters, ACT 1 of 3
                x1 = xp.tile([128, FD], F16, name="x1", tag="x1")
                if i % 3 == 2:
                    nc.scalar.activation(x1[:], ps1[:], Relu, bias=bg1dd_sb[:])
                else:
                    nc.vector.tensor_scalar(
                        x1[:], ps1[:], bg1dd_sb[:], 0.0, op0=add, op1=mx
                    )
                # layer 2 (note: output halves land swapped; harmless for sum)
                ps2 = ps2p.tile([128, FD], F32, name="ps2", tag="ps2")
                for c in range(FD // 512):
                    cs = slice(512 * c, 512 * (c + 1))
                    nc.tensor.matmul(
                        ps2[64:128, cs], w2dd_sb[0:64, :], x1[0:64, cs],
                        start=True, stop=True,
                    )
                    nc.tensor.matmul(
                        ps2[0:64, cs], w2dd_sb[64:128, :], x1[64:128, cs],
                        start=True, stop=True,
                    )
                # X2 = relu(ps2 + bg2) on ACT; accum_out -> row sums
                x2 = scratch.tile([128, FD], F16, name="x2", tag="x2")
                nc.scalar.activation(
                    x2[:], ps2[:], Relu, bias=bg2dd_sb[:],
                    accum_out=acc[:, i : i + 1],
                )

            # Reduce accumulated columns -> [128, 1]
            accr = setup.tile([128, 1], F32, name="accr", tag="accr")
            nc.vector.tensor_reduce(
                accr[:], acc[:], axis=mybir.AxisListType.X, op=add
            )
            # f-network; K=128 matmul folds top+bottom halves of accr
            ps_h = pssp.tile([F, 1], F32, name="ps_h", tag="ps1")
            nc.tensor.matmul(ps_h[:], wpT_dd_sb[:], accr[:], start=True, stop=True)
            h_sb = setup.tile([F, 1], F32, name="h_sb", tag="h_sb")
            nc.scalar.activation(h_sb[:], ps_h[:], Relu, bias=bp_sb[:])
            ps_o = pssp.tile([F, 1], F32, name="ps_o", tag="ps1")
            nc.tensor.matmul(ps_o[:], woT_sb[:], h_sb[:], start=True, stop=True)
            o_sb = setup.tile([F, 1], F32, name="o_sb", tag="o_sb")
            nc.scalar.activation(o_sb[:], ps_o[:], Ident, bias=bo_sb[:])
            nc.sync.dma_start(out[b], o_sb[:])


def _shared_in_map(Wg0, bg0, Wg1, bg1, Wg2, bg2, Wp, bp, Wo, bo):
    f = np.float32
    wg0l = np.ascontiguousarray(Wg0[:, :D].T, dtype=f)  # [65, 64]
    wg0r = np.ascontiguousarray(Wg0[:, D:].T, dtype=f)  # [65, 64]
    stackT = lambda w: np.concatenate(
        [np.ascontiguousarray(w.T, dtype=f)] * 2, axis=0
    )
    return {
        "coord": np.arange(L, dtype=f).reshape(1, L),
        "wg0lT_dd": np.concatenate([wg0l, wg0l], axis=1),
        "wg0rT_dd": np.concatenate([wg0r, wg0r], axis=1),
        "bg0dd": np.concatenate([bg0, bg0]).astype(f).reshape(128, 1),
        "w1dd": stackT(Wg1).astype(np.float16),
        "w2dd": stackT(Wg2).astype(np.float16),
        "bg1dd": np.concatenate([bg1, bg1]).astype(f).reshape(128, 1),
        "bg2dd": np.concatenate([bg2, bg2]).astype(f).reshape(128, 1),
        "wpT_dd": np.concatenate([Wp.T, Wp.T], axis=0).astype(f),
        "bp_c": np.asarray(bp, f).reshape(F, 1),
        "woT": np.ascontiguousarray(Wo.T, dtype=f),
        "bo_c": np.asarray(bo, f).reshape(F, 1),
    }


def kernel(
    x_img, Wg0, bg0, Wg1, bg1, Wg2, bg2, Wp, bp, Wo, bo, trace=False, **run_kwargs
):
    if "nc" not in _CACHE:
        _CACHE["nc"] = _build_nc()
    nc = _CACHE["nc"]

    shared = _shared_in_map(
        np.asarray(Wg0), np.asarray(bg0), np.asarray(Wg1), np.asarray(bg1),
        np.asarray(Wg2), np.asarray(bg2), np.asarray(Wp), np.asarray(bp),
        np.asarray(Wo), np.asarray(bo),
    )
    x = np.asarray(x_img, dtype=np.float32)
    bsz = x.shape[0]
    x = x.reshape(bsz, C, L)

    in_maps = []
    for core in range(N_CORES):
        m = dict(shared)
        m["xf"] = np.ascontiguousarray(x[core * B_PER_CORE : (core + 1) * B_PER_CORE])
        in_maps.append(m)

    res = run_bass_kernel_spmd(
        nc, in_maps, core_ids=list(range(N_CORES)), trace=trace, **run_kwargs
    )
    outs = [r["out"].reshape(B_PER_CORE, F) for r in res.results]
    full = np.concatenate(outs, axis=0)
    if trace:
        _CACHE["last_results"] = res
    return full
